# revision 6
# baseline (speedup 1.0000x reference)
"""CenterLoss kernel for Trainium2 (8 NeuronCores, Bass/Tile).

Computation (reference):
    h = prelu(x, a)                      # [B, D]
    output = h @ fc3_w.T + fc3_b         # [B, C]
    c = centers[labels]                  # [B, D]
    dist = clip(sum((x - c)^2, -1), 1e-12, 1e12)
    loss = mean(dist)

Sharding: fc3_w / fc3_b are split along the class axis C across the 8
cores (tensor parallel over classes); x and the gathered center rows are
replicated.  Each core computes its [B, C/8] slice of the logits; the
(tiny) center-loss branch is computed redundantly on every core and the
result is taken from core 0.  The label gather over the full `centers`
table is an indexing-only host op; only the gathered [B, D] rows are
shipped to the device (per the sharding hint: "all-gather only the
per-label gathered center rows").

Device layout: x and the gathered centers are shipped pre-transposed
[D, B] so the contraction dim D=128 sits on SBUF partitions for both
matmul operands; fc3_w is shipped as w.T [D, C/8] for the same reason.

Perf notes (from NTFF profiles):
 - fp32 matmul runs at 4 cycles/row; float32r (same bits, reduced-
   precision multiply) runs at 1 cycle/row for free dim >= 256.  The
   logits matmul uses float32r; the loss branch stays full fp32.
 - The bias add rides the PE as a K=1 accumulate-matmul into the same
   PSUM group, so the PSUM->SBUF drain is a pure copy that can be split
   between the Vector and Scalar engines.
 - Output writes (102 MB/core) saturate HBM write bandwidth; inputs load
   via the SWDGE (gpsimd) ring so the HWDGE ring is dedicated to output.
"""

import numpy as np

B, C, D = 2048, 100000, 128
NCORES = 8
CS = C // NCORES          # 12500 classes per core
NCHUNK = 500              # matmul moving free dim (<=512 fp32, divides CS)
CHUNKS = CS // NCHUNK     # 25
GROUP = 5                 # psum chunks per output staging tile
OUTW = NCHUNK * GROUP     # 2500 columns per output DMA
MT = B // 128             # 16 row tiles
CLAMP_MIN, CLAMP_MAX = 1e-12, 1e12

USE_F32R = True

_CACHE = {}


def _build(alpha: float):
    import concourse.tile as tile
    import concourse.mybir as mybir
    from concourse import bacc

    f32 = mybir.dt.float32
    f32r = mybir.dt.float32r
    Alu = mybir.AluOpType

    mdt = f32r if USE_F32R else f32

    nc = bacc.Bacc(
        "TRN2",
        target_bir_lowering=False,
        debug=False,
        enable_asserts=True,
        num_devices=NCORES,
    )
    xT_d = nc.dram_tensor("xT", [D, B], f32, kind="ExternalInput").ap()
    cT_d = nc.dram_tensor("cT", [D, B], f32, kind="ExternalInput").ap()
    wT_d = nc.dram_tensor("wT", [D, CS], f32, kind="ExternalInput").ap()
    bias_d = nc.dram_tensor("bias", [1, CS], f32, kind="ExternalInput").ap()
    out_d = nc.dram_tensor("out", [B, CS], f32, kind="ExternalOutput").ap()
    loss_d = nc.dram_tensor("loss", [1, 1], f32, kind="ExternalOutput").ap()

    with tile.TileContext(nc) as tc:
        with (
            tc.tile_pool(name="persist", bufs=1) as persist,
            tc.tile_pool(name="outp", bufs=4) as outp,
            tc.tile_pool(name="mm_psum", bufs=6, space="PSUM") as mm_psum,
            tc.tile_pool(name="ls_psum", bufs=2, space="PSUM") as ls_psum,
            tc.tile_pool(name="small", bufs=1) as small,
        ):
            xT = persist.tile([D, B], f32, tag="xT")
            nc.gpsimd.dma_start(xT[:], xT_d[:, :])
            cT = persist.tile([D, B], f32, tag="cT")
            nc.gpsimd.dma_start(cT[:], cT_d[:, :])
            # f32r operands must be produced pre-rounded; the SWDGE DMA
            # casts f32 -> f32r inline.
            bias = persist.tile([1, CS], mdt, tag="bias")
            nc.gpsimd.dma_start(bias[:], bias_d[:, :])
            wT = persist.tile([D, CS], mdt, tag="wT")
            # Load weights in column groups so the first matmuls can start
            # without waiting for the whole 6.4 MB transfer.
            for g in range(CHUNKS // GROUP):
                sl = slice(g * OUTW, (g + 1) * OUTW)
                nc.gpsimd.dma_start(wT[:, sl], wT_d[:, sl])

            ones_f = small.tile([1, 128], f32, tag="ones_f")
            nc.vector.memset(ones_f[:], 1.0)
            ones = small.tile([1, 128], mdt, tag="ones")
            nc.vector.tensor_copy(out=ones[:], in_=ones_f[:])

            # hT = prelu(xT) = max(x, 0) + alpha * min(x, 0)
            hT = persist.tile([D, B], mdt, tag="hT")
            tpos = small.tile([D, B], f32, tag="tpos")
            tneg = small.tile([D, B], f32, tag="tneg")
            nc.vector.tensor_scalar(tneg[:], xT[:], 0.0, alpha, Alu.min, Alu.mult)
            nc.vector.tensor_scalar_max(tpos[:], xT[:], 0.0)
            nc.vector.tensor_add(hT[:], tneg[:], tpos[:])

            # ---- center-loss branch (full fp32) ----
            sq = small.tile([D, B], f32, tag="sq")
            nc.vector.tensor_tensor(sq[:], xT[:], cT[:], Alu.subtract)
            nc.vector.tensor_mul(sq[:], sq[:], sq[:])
            onesc = small.tile([128, 1], f32, tag="onesc")
            nc.vector.memset(onesc[:], 1.0)
            dist = small.tile([1, B], f32, tag="dist")
            for q in range(B // 512):
                ps = ls_psum.tile([1, 512], f32, tag="lps")
                nc.tensor.matmul(
                    ps[:],
                    onesc[:, 0:1],
                    sq[:, q * 512:(q + 1) * 512],
                    start=True,
                    stop=True,
                )
                nc.vector.tensor_copy(out=dist[0:1, q * 512:(q + 1) * 512], in_=ps[:])
            nc.vector.tensor_scalar(
                dist[:], dist[:], CLAMP_MIN, CLAMP_MAX, Alu.max, Alu.min
            )
            lsum = small.tile([1, 1], f32, tag="lsum")
            nc.vector.reduce_sum(lsum[0:1, 0:1], dist[0:1, :], axis=mybir.AxisListType.X)
            nc.vector.tensor_scalar_mul(lsum[:], lsum[:], 1.0 / B)
            nc.sync.dma_start(loss_d[:, :], lsum[:])

            # ---- classifier branch: out = h @ w.T + bias ----
            for t in range(MT):
                lhsT = hT[:, t * 128:(t + 1) * 128]
                for g in range(CHUNKS // GROUP):
                    ot = outp.tile([128, OUTW], f32, tag="ot")
                    for j in range(GROUP):
                        n0 = (g * GROUP + j) * NCHUNK
                        ps = mm_psum.tile([128, NCHUNK], f32, tag="mmps")
                        nc.tensor.matmul(
                            ps[:],
                            lhsT,
                            wT[:, n0:n0 + NCHUNK],
                            start=True,
                            stop=False,
                        )
                        nc.tensor.matmul(
                            ps[:],
                            ones[0:1, :],
                            bias[0:1, n0:n0 + NCHUNK],
                            start=False,
                            stop=True,
                        )
                        dst = ot[:, j * NCHUNK:(j + 1) * NCHUNK]
                        if j % 2 == 0:
                            nc.vector.tensor_copy(out=dst, in_=ps[:])
                        else:
                            nc.scalar.copy(out=dst, in_=ps[:])
                    nc.sync.dma_start(
                        out_d[t * 128:(t + 1) * 128, g * OUTW:(g + 1) * OUTW],
                        ot[:],
                    )

    nc.compile()
    return nc


def _run(inputs, trace=False, trace_cores=None):
    from concourse.bass_utils import run_bass_kernel_spmd

    x = np.ascontiguousarray(np.asarray(inputs["x"], dtype=np.float32))
    centers = np.asarray(inputs["centers"], dtype=np.float32)
    prelu_a = np.asarray(inputs["prelu_a"], dtype=np.float32)
    fc3_w = np.asarray(inputs["fc3_w"], dtype=np.float32)
    fc3_b = np.asarray(inputs["fc3_b"], dtype=np.float32)
    labels = np.asarray(inputs["labels"])

    alpha = float(prelu_a.reshape(-1)[0])

    xT = np.ascontiguousarray(x.T)                       # [D, B]
    cT = np.ascontiguousarray(centers[labels].T)         # [D, B]

    in_maps = []
    for m in range(NCORES):
        wm = fc3_w[m * CS:(m + 1) * CS, :]               # [CS, D]
        bm = fc3_b[m * CS:(m + 1) * CS]                  # [CS]
        in_maps.append({
            "xT": xT,
            "cT": cT,
            "wT": np.ascontiguousarray(wm.T),            # [D, CS]
            "bias": np.ascontiguousarray(bm.reshape(1, CS)),
        })

    key = alpha
    if key not in _CACHE:
        _CACHE[key] = _build(alpha)
    nc = _CACHE[key]

    res = run_bass_kernel_spmd(
        nc,
        in_maps,
        core_ids=list(range(NCORES)),
        trace=trace,
        trace_cores=trace_cores,
    )
    output = np.concatenate([r["out"] for r in res.results], axis=1)
    loss = np.asarray(res.results[0]["loss"], dtype=np.float32).reshape(())
    return (loss, output), res


def kernel(**inputs):
    (loss, output), _ = _run(inputs, trace=False)
    return loss, output


# revision 8
# speedup vs baseline: 1.7518x; 1.7518x over previous
"""CenterLoss kernel for Trainium2 (8 NeuronCores, Bass/Tile).

Computation (reference):
    h = prelu(x, a)                      # [B, D]
    output = h @ fc3_w.T + fc3_b         # [B, C]
    c = centers[labels]                  # [B, D]
    dist = clip(sum((x - c)^2, -1), 1e-12, 1e12)
    loss = mean(dist)

Sharding: fc3_w / fc3_b are split along the class axis C across the 8
cores (tensor parallel over classes); x and the gathered center rows are
replicated.  Each core computes its [B, C/8] slice of the logits; the
(tiny) center-loss branch is computed redundantly on every core and the
result is taken from core 0.  The label gather over the full `centers`
table is an indexing-only host op; only the gathered [B, D] rows are
shipped to the device (per the sharding hint: "all-gather only the
per-label gathered center rows").

Device layout: x and the gathered centers are shipped pre-transposed
[D, B] so the contraction dim D=128 sits on SBUF partitions for both
matmul operands; fc3_w is shipped as w.T [D, C/8] for the same reason.

Perf notes (from NTFF profiles on this part):
 - fp32 matmul = 2 half-rate passes; float32r (same bits, reduced-
   precision multiply) is a single pass, ~1.8x faster measured.  The
   logits matmul uses float32r (absmax-rel err ~1.5e-4 vs fp32 ref);
   the loss branch stays full fp32.
 - PSUM has no DMA route, so each logit chunk is drained PSUM->SBUF by
   a copy; chunks are paired into 2-bank PSUM tiles and the drains
   alternate between the Vector and Scalar engines.
 - fc3_b is all-zero for this problem's inputs; the build is
   specialized at runtime (checked on host).  A nonzero bias falls back
   to a build that adds bias during the Vector-engine drain.
 - Output writes (102 MB/core) are the roofline (~360 GB/s/core HBM);
   inputs load via the SWDGE (gpsimd) ring so the HWDGE ring is
   dedicated to output.
"""

import numpy as np

B, C, D = 2048, 100000, 128
NCORES = 8
CS = C // NCORES          # 12500 classes per core
NCHUNK = 500              # matmul moving free dim (<=512 fp32, divides CS)
CHUNKS = CS // NCHUNK     # 25
GROUP = 5                 # chunks per output staging tile
OUTW = NCHUNK * GROUP     # 2500 columns per output DMA
MT = B // 128             # 16 row tiles
CLAMP_MIN, CLAMP_MAX = 1e-12, 1e12

USE_F32R = True

_CACHE = {}


def _build(alpha: float, zero_bias: bool):
    import concourse.tile as tile
    import concourse.mybir as mybir
    from concourse import bacc

    f32 = mybir.dt.float32
    f32r = mybir.dt.float32r
    Alu = mybir.AluOpType
    mdt = f32r if USE_F32R else f32

    nc = bacc.Bacc(
        "TRN2",
        target_bir_lowering=False,
        debug=False,
        enable_asserts=True,
        num_devices=NCORES,
    )
    xT_d = nc.dram_tensor("xT", [D, B], f32, kind="ExternalInput").ap()
    cT_d = nc.dram_tensor("cT", [D, B], f32, kind="ExternalInput").ap()
    wT_d = nc.dram_tensor("wT", [D, CS], f32, kind="ExternalInput").ap()
    bias_d = nc.dram_tensor("bias", [1, CS], f32, kind="ExternalInput").ap()
    out_d = nc.dram_tensor("out", [B, CS], f32, kind="ExternalOutput").ap()
    loss_d = nc.dram_tensor("loss", [1, 1], f32, kind="ExternalOutput").ap()

    with tile.TileContext(nc) as tc:
        with (
            tc.tile_pool(name="persist", bufs=1) as persist,
            tc.tile_pool(name="outp", bufs=4) as outp,
            tc.tile_pool(name="mm_psum", bufs=3, space="PSUM") as mm_psum,
            tc.tile_pool(name="ls_psum", bufs=2, space="PSUM") as ls_psum,
            tc.tile_pool(name="small", bufs=1) as small,
        ):
            xT = persist.tile([D, B], f32, tag="xT")
            nc.gpsimd.dma_start(xT[:], xT_d[:, :])
            cT = persist.tile([D, B], f32, tag="cT")
            nc.gpsimd.dma_start(cT[:], cT_d[:, :])
            # f32r operands must be produced pre-rounded; the SWDGE DMA
            # casts f32 -> f32r inline.
            wT = persist.tile([D, CS], mdt, tag="wT")
            # Load weights in column groups so the first matmuls can start
            # without waiting for the whole 6.4 MB transfer.
            for g in range(CHUNKS // GROUP):
                sl = slice(g * OUTW, (g + 1) * OUTW)
                nc.gpsimd.dma_start(wT[:, sl], wT_d[:, sl])

            if not zero_bias:
                # bias broadcast across partitions, built on device via a
                # K=1 ones x bias matmul (one-time cost).
                bias_row = small.tile([1, CS], mdt, tag="bias_row")
                nc.gpsimd.dma_start(bias_row[:], bias_d[:, :])
                ones_f = small.tile([1, 128], f32, tag="ones_f")
                nc.vector.memset(ones_f[:], 1.0)
                ones = small.tile([1, 128], mdt, tag="ones")
                nc.vector.tensor_copy(out=ones[:], in_=ones_f[:])
                bb = persist.tile([128, CS], f32, tag="bb")
                for q in range(CHUNKS):
                    n0 = q * NCHUNK
                    psb = ls_psum.tile([128, NCHUNK], f32, tag="bbps")
                    nc.tensor.matmul(
                        psb[:],
                        ones[0:1, :],
                        bias_row[0:1, n0:n0 + NCHUNK],
                        start=True,
                        stop=True,
                    )
                    nc.vector.tensor_copy(out=bb[:, n0:n0 + NCHUNK], in_=psb[:])
            else:
                bb = None

            # hT = prelu(xT) = max(x, 0) + alpha * min(x, 0)
            hT = persist.tile([D, B], mdt, tag="hT")
            tpos = small.tile([D, B], f32, tag="tpos")
            tneg = small.tile([D, B], f32, tag="tneg")
            nc.vector.tensor_scalar(tneg[:], xT[:], 0.0, alpha, Alu.min, Alu.mult)
            nc.vector.tensor_scalar_max(tpos[:], xT[:], 0.0)
            nc.vector.tensor_add(hT[:], tneg[:], tpos[:])

            # ---- center-loss branch (full fp32) ----
            sq = small.tile([D, B], f32, tag="sq")
            nc.vector.tensor_tensor(sq[:], xT[:], cT[:], Alu.subtract)
            nc.vector.tensor_mul(sq[:], sq[:], sq[:])
            onesc = small.tile([128, 1], f32, tag="onesc")
            nc.vector.memset(onesc[:], 1.0)
            dist = small.tile([1, B], f32, tag="dist")
            for q in range(B // 512):
                ps = ls_psum.tile([1, 512], f32, tag="lps")
                nc.tensor.matmul(
                    ps[:],
                    onesc[:, 0:1],
                    sq[:, q * 512:(q + 1) * 512],
                    start=True,
                    stop=True,
                )
                nc.vector.tensor_copy(out=dist[0:1, q * 512:(q + 1) * 512], in_=ps[:])
            nc.vector.tensor_scalar(
                dist[:], dist[:], CLAMP_MIN, CLAMP_MAX, Alu.max, Alu.min
            )
            lsum = small.tile([1, 1], f32, tag="lsum")
            nc.vector.reduce_sum(lsum[0:1, 0:1], dist[0:1, :], axis=mybir.AxisListType.X)
            nc.vector.tensor_scalar_mul(lsum[:], lsum[:], 1.0 / B)
            nc.sync.dma_start(loss_d[:, :], lsum[:])

            # ---- classifier branch: out = h @ w.T (+ bias) ----
            # Chunks are paired into one 2-bank PSUM tile; the drain to the
            # output staging tile alternates Vector / Scalar engines.
            drain_i = 0
            for t in range(MT):
                lhsT = hT[:, t * 128:(t + 1) * 128]
                for g in range(CHUNKS // GROUP):
                    ot = outp.tile([128, OUTW], f32, tag="ot")
                    for jp in range((GROUP + 1) // 2):
                        j0 = jp * 2
                        npair = min(2, GROUP - j0)
                        w_cols = NCHUNK * npair
                        n0 = (g * GROUP + j0) * NCHUNK
                        # one bank-aligned 512-wide sub-tile per chunk so no
                        # matmul output crosses a PSUM bank boundary
                        ps = mm_psum.tile([128, 2, 512], f32, tag="mmps")
                        for jj in range(npair):
                            nc.tensor.matmul(
                                ps[:, jj, :NCHUNK],
                                lhsT,
                                wT[:, n0 + jj * NCHUNK:n0 + (jj + 1) * NCHUNK],
                                start=True,
                                stop=True,
                            )
                        if npair == 2:
                            dst = ot[:, j0 * NCHUNK:j0 * NCHUNK + w_cols].rearrange(
                                "p (b e) -> p b e", e=NCHUNK
                            )
                            src = ps[:, :, :NCHUNK]
                        else:
                            dst = ot[:, j0 * NCHUNK:j0 * NCHUNK + w_cols]
                            src = ps[:, 0, :NCHUNK]
                        if zero_bias:
                            if drain_i % 2 == 0:
                                nc.vector.tensor_copy(out=dst, in_=src)
                            else:
                                nc.scalar.copy(out=dst, in_=src)
                        else:
                            nc.vector.tensor_tensor(
                                dst, src, bb[:, n0:n0 + w_cols], Alu.add
                            )
                        drain_i += 1
                    nc.sync.dma_start(
                        out_d[t * 128:(t + 1) * 128, g * OUTW:(g + 1) * OUTW],
                        ot[:],
                    )

    nc.compile()
    return nc


def _run(inputs, trace=False, trace_cores=None):
    from concourse.bass_utils import run_bass_kernel_spmd

    x = np.ascontiguousarray(np.asarray(inputs["x"], dtype=np.float32))
    centers = np.asarray(inputs["centers"], dtype=np.float32)
    prelu_a = np.asarray(inputs["prelu_a"], dtype=np.float32)
    fc3_w = np.asarray(inputs["fc3_w"], dtype=np.float32)
    fc3_b = np.asarray(inputs["fc3_b"], dtype=np.float32)
    labels = np.asarray(inputs["labels"])

    alpha = float(prelu_a.reshape(-1)[0])
    zero_bias = not np.any(fc3_b)

    xT = np.ascontiguousarray(x.T)                       # [D, B]
    cT = np.ascontiguousarray(centers[labels].T)         # [D, B]

    in_maps = []
    for m in range(NCORES):
        wm = fc3_w[m * CS:(m + 1) * CS, :]               # [CS, D]
        bm = fc3_b[m * CS:(m + 1) * CS]                  # [CS]
        in_maps.append({
            "xT": xT,
            "cT": cT,
            "wT": np.ascontiguousarray(wm.T),            # [D, CS]
            "bias": np.ascontiguousarray(bm.reshape(1, CS)),
        })

    key = (alpha, zero_bias)
    if key not in _CACHE:
        _CACHE[key] = _build(alpha, zero_bias)
    nc = _CACHE[key]

    res = run_bass_kernel_spmd(
        nc,
        in_maps,
        core_ids=list(range(NCORES)),
        trace=trace,
        trace_cores=trace_cores,
    )
    output = np.concatenate([r["out"] for r in res.results], axis=1)
    loss = np.asarray(res.results[0]["loss"], dtype=np.float32).reshape(())
    return (loss, output), res


def kernel(**inputs):
    (loss, output), _ = _run(inputs, trace=False)
    return loss, output
